# revision 27
# baseline (speedup 1.0000x reference)
"""Trainium2 Bass kernel for nn_BiAttention (MoE-routed bi-attention).

Strategy (8 NeuronCores, SPMD single program):
- Batches are assigned to 4 per-core slots by a host-side optimizer that
  minimizes per-slot expert capacities; core c, slot bl holds batch
  ASN[bl][c].  Within each slot the 512 tokens are stable-sorted by expert
  and zero-padded to per-slot capacities cap[bl][e] (max over the 8 cores
  sharing the slot), giving per-slot padded length L[bl] (~574-588).
- Projections run in fp8e4 with DoubleRow perf mode (two stacked k-tiles
  per instruction) as a 3-term residual sum W8@x8 + W8@xr8 + Wr8@x8, which
  keeps the quantization error of both x and W at fp16-like levels while
  paying only 1.5x the single-term fp8 cost.  Weights are pre-scaled by 32
  on the host to sit in e4m3's normal range.
- Q/K land in a "paired" fp8 layout [32*j+p, hg, tk, tok] (head hg*4+j,
  dk = 32*tk + p) so the dk=64 score contraction runs as one DoubleRow
  matmul per key-slab.  V lands in bf16 (dk-major), is PE-transposed per
  128-token key-slab into token-major v_sb[tok, bl, ks, h, 65] with
  column 64 preset to 32.0 so P@V emits the softmax denominator Z free.
- scoresT[k, q] per key-slab.  exp: the first KS-2 key-slabs run on the
  Activation engine (per-partition key-mask bias) writing bf16 E; the
  last 2 run a Schraudolph exponent-field exp on DVE (one fused
  mult+add into int32 whose high half IS the bf16 result - P@V reads a
  strided bitcast view, no convert pass).  P@V runs plain bf16 with E
  stationary producing O token-major [q, qs, 65]; normalization =
  reciprocal of Z (DVE) + broadcast multiply (Pool); outputs DMA
  token-major bf16; the host un-permutes.
- Ordering: V projections and transposes first, then Q/K for head-group
  0, then head-group 1, with attention heads 0-3 scheduled before 4-7 and
  all PSUM tags in one shared pool, so attention overlaps the tail of the
  projection phase.
"""
import numpy as np
import ml_dtypes

import concourse.bass as bass
import concourse.mybir as mybir
import concourse.tile as tile
from concourse.bass_utils import run_bass_kernel_spmd

F32 = mybir.dt.float32
BF16 = mybir.dt.bfloat16
FP8 = mybir.dt.float8e4
I32 = mybir.dt.int32
DR = mybir.MatmulPerfMode.DoubleRow
EXP = mybir.ActivationFunctionType.Exp

B, N, D, H, DK, NE = 32, 512, 512, 8, 64, 9
NCORES = 8
BL = B // NCORES        # slots per core
W_SCALE = 32.0          # host pre-scale of all weights (fp8 range)
ACT_SCALE = 1.0 / (8.0 * W_SCALE * W_SCALE)  # 1/sqrt(dk) / (32*32)
# Schraudolph exp (DVE offload of the Activation engine): the last N_SCH
# key-slabs of each head compute exp via the exponent-field bit trick.
N_SCH = 2
SCH_A = float(ACT_SCALE * (1 << 23) / np.log(2.0))
SCH_C = 405000.0        # tuned for min relative rmse after bf16 truncation
SCH_B = float((127 << 23) - SCH_C)
SCH_BIAS_PAD = float(-30.0 * (1 << 23) / np.log(2.0))   # masked keys -> ~0

ENGINE_OK = {
    mybir.EngineType.PE,
    mybir.EngineType.Activation,
    mybir.EngineType.DVE,
    mybir.EngineType.Pool,
    mybir.EngineType.SP,
}


def _fix_multiwait(nc, cap_default=1, cap_evsem=2):
    """walrus in this container accepts at most 1 sync-wait per instruction;
    move excess waits onto freshly inserted same-engine NoOps."""
    uid = 0
    for fn in nc.m.functions:
        for bb in fn.blocks:
            insts = bb.instructions
            i = 0
            while i < len(insts):
                ins = insts[i]
                si = getattr(ins, "sync_info", None)
                waits = list(si.on_wait) if (si and si.on_wait) else []
                cap = cap_evsem if isinstance(ins, mybir.InstEventSemaphore) else cap_default
                if len(waits) > cap and ins.engine in ENGINE_OK:
                    extra, keep = waits[:-cap], waits[-cap:]
                    si.on_wait = keep
                    nops = []
                    for w in extra:
                        uid += 1
                        nops.append(mybir.InstNoOp(
                            name=f"I-mwfix-{uid}",
                            engine=ins.engine,
                            ins=[], outs=[],
                            sync_info=mybir.SyncInfo(on_wait=[w], on_update=[]),
                            text_hint="multiwait_fix",
                        ))
                    insts[i:i] = nops
                    i += len(nops)
                i += 1


def _seg_chunks(s, n, bank_elems=512):
    """Split [s, s+n) at multiples of bank_elems (psum bank boundaries)."""
    out = []
    cur, end = s, s + n
    while cur < end:
        nxt = min(end, (cur // bank_elems + 1) * bank_elems)
        out.append((cur, nxt - cur))
        cur = nxt
    return out


def _build_program(caps, starts, Ls):
    """caps/starts: [BL][NE] per-slot capacities and segment starts;
    Ls: [BL] per-slot padded lengths."""
    KSs = [-(-L // 128) for L in Ls]
    offs = [0]
    for L in Ls:
        offs.append(offs[-1] + L)
    TC = offs[-1]                       # total columns per core
    TCP = -(-TC // 16) * 16             # DoubleRow: 16B-aligned tile strides
    LMAX = max(Ls)
    KSMAX = max(KSs)
    QSMAX = -(-LMAX // 128)

    nc = bass.Bass()
    x_d = [[nc.dram_tensor(f"x{s + 1}", [128, 4, TCP], FP8,
                           kind="ExternalInput"),
            nc.dram_tensor(f"xr{s + 1}", [128, 4, TCP], FP8,
                           kind="ExternalInput")] for s in range(2)]
    # wqk[si]: [2i, 2hg, 2tk, NE, 2r, 128p, 4ks, 128m]
    wqk_d = [nc.dram_tensor(f"wqk{s}", [2, 2, 2, NE, 2, 128, 4, 128], FP8,
                            kind="ExternalInput") for s in range(2)]
    # wv[si]: [4ms, NE, 2r, 128p, 4ks, 128m]
    wv_d = [nc.dram_tensor(f"wv{s}", [4, NE, 2, 128, 4, 128], FP8,
                           kind="ExternalInput") for s in range(2)]
    bias_d = nc.dram_tensor("bias", [128, BL, KSMAX], F32, kind="ExternalInput")
    bias2_d = nc.dram_tensor("bias2", [128, BL, KSMAX], F32,
                             kind="ExternalInput")
    id_d = nc.dram_tensor("iden", [128, 128], BF16, kind="ExternalInput")
    o_d = nc.dram_tensor("o", [2, BL, QSMAX * 128, 512], BF16,
                         kind="ExternalOutput")

    with tile.TileContext(nc) as tc:
        with (
            tc.tile_pool(name="const", bufs=1) as constp,
            tc.tile_pool(name="qk", bufs=1) as qkp,
            tc.tile_pool(name="vsb", bufs=1) as vp,
            tc.tile_pool(name="xp", bufs=1) as xp,
            tc.tile_pool(name="vtp", bufs=1) as vtp,
            tc.tile_pool(name="wp", bufs=2) as wp,
            tc.tile_pool(name="ep", bufs=2) as ep,
            tc.tile_pool(name="eip", bufs=4) as eip,
            tc.tile_pool(name="op", bufs=3) as op_,
            tc.tile_pool(name="rp", bufs=2) as rp,
            tc.tile_pool(name="pp", bufs=1, space="PSUM") as ppool,
        ):
            id_sb = constp.tile([128, 128], BF16)
            nc.sync.dma_start(id_sb[:], id_d[:])
            bias_sb = constp.tile([128, BL, KSMAX], F32)
            nc.sync.dma_start(bias_sb[:], bias_d[:])
            bias2_sb = constp.tile([128, BL, KSMAX], F32)
            nc.sync.dma_start(bias2_sb[:], bias2_d[:])

            # persistent paired Q/K: [128(4h x 32dk), hg, tk, TC] per side
            qt = [qkp.tile([128, 2, 2, TCP], FP8, tag=f"qt{s}", name=f"qt{s}")
                  for s in range(2)]
            kt = [qkp.tile([128, 2, 2, TCP], FP8, tag=f"kt{s}", name=f"kt{s}")
                  for s in range(2)]
            # token-major V: [128(tok), bl, ks, h, 65], col 64 = 32.0 (Z trick)
            v_sb = [vp.tile([128, BL, KSMAX, H, DK + 1], BF16,
                            tag=f"v{s}", name=f"v{s}") for s in range(2)]
            for s in range(2):
                nc.vector.memset(v_sb[s][:, :, :, :, DK:DK + 1], 32.0)

            x8s, xrs = [], []
            for si in range(2):
                x8_sb = xp.tile([128, 4, TCP], FP8, tag=f"x8{si}",
                                name=f"x8_{si}")
                nc.sync.dma_start(x8_sb[:], x_d[si][0][:])
                xr_sb = xp.tile([128, 4, TCP], FP8, tag=f"xr{si}",
                                name=f"xr_{si}")
                nc.sync.dma_start(xr_sb[:], x_d[si][1][:])
                x8s.append(x8_sb)
                xrs.append(xr_sb)

            eng_i = 0

            def copy(dst, src):
                nonlocal eng_i
                eng_i += 1
                if eng_i % 2 == 0:
                    nc.scalar.copy(dst, src)
                else:
                    nc.vector.tensor_copy(dst, src)

            def proj_pass(w_sb, si, bl):
                """3-term fp8 DoubleRow psum pass over all experts of one
                slot. w_sb: [128, NE, 2r, 4ks, 128m]. Returns psum tile."""
                pq = ppool.tile([128, LMAX], F32, tag="big",
                                padded_shape=[128, 1024], bufs=3)
                for e in range(NE):
                    ce = caps[bl][e]
                    if ce == 0:
                        continue
                    for (s0, sn) in _seg_chunks(starts[bl][e], ce):
                        terms = ((0, x8s[si]), (0, xrs[si]), (1, x8s[si]))
                        for ti, (r, xs) in enumerate(terms):
                            for dp in range(2):
                                nc.tensor.matmul(
                                    pq[:, s0:s0 + sn],
                                    w_sb[:, e, r, 2 * dp:2 * dp + 2, :],
                                    xs[:, 2 * dp:2 * dp + 2,
                                       offs[bl] + s0:offs[bl] + s0 + sn],
                                    start=(ti == 0 and dp == 0),
                                    stop=(ti == 2 and dp == 1),
                                    perf_mode=DR,
                                )
                return pq

            # ---- V projections + transposes (both sides) ----
            for si in range(2):
                vt_sb = vtp.tile([128, 4, TCP], BF16, tag="vt")
                for ms in range(4):
                    w_sb = wp.tile([128, NE, 2, 4, 128], FP8, tag="wv")
                    nc.sync.dma_start(
                        w_sb[:],
                        wv_d[si][ms].rearrange("e r p k m -> p e r k m"))
                    for bl in range(BL):
                        pq = proj_pass(w_sb, si, bl)
                        copy(vt_sb[:, ms, offs[bl]:offs[bl] + Ls[bl]],
                             pq[:, 0:Ls[bl]])
                for bl in range(BL):
                    L, KS = Ls[bl], KSs[bl]
                    for ksl in range(KS):
                        sz = min(128, L - ksl * 128)
                        tv = ppool.tile([128, 4, 128], BF16, tag="small",
                                        bufs=2)
                        for ms in range(4):
                            nc.tensor.transpose(
                                tv[0:sz, ms, :],
                                vt_sb[:, ms, offs[bl] + ksl * 128:
                                      offs[bl] + ksl * 128 + sz],
                                id_sb[:],
                            )
                        src = tv[0:sz, :, :].rearrange(
                            "k m (h2 dd) -> k (m h2) dd", h2=2)
                        copy(v_sb[si][0:sz, bl, ksl, :, 0:DK], src)

            # ---- Q/K projections, head-group 0 first ----
            for hg in range(2):
                for si in range(2):
                    for i, dst in enumerate((qt[si], kt[si])):
                        for tk in range(2):
                            w_sb = wp.tile([128, NE, 2, 4, 128], FP8,
                                           tag="wqk")
                            nc.sync.dma_start(
                                w_sb[:],
                                wqk_d[si][i, hg, tk].rearrange(
                                    "e r p k m -> p e r k m"))
                            for bl in range(BL):
                                pq = proj_pass(w_sb, si, bl)
                                copy(dst[:, hg, tk,
                                         offs[bl]:offs[bl] + Ls[bl]],
                                     pq[:, 0:Ls[bl]])

            # ---- attention, head-group 0 first ----
            for hg in range(2):
                for bl in range(BL):
                    L, KS = Ls[bl], KSs[bl]
                    QS = -(-L // 128)
                    qch = _seg_chunks(0, L)
                    for att in range(2):
                        qs_side = 1 - att   # h1: Q from type side (x2)
                        kv_side = att
                        o_sb = op_.tile([128, QSMAX, 4, DK], BF16, tag="o")
                        pvs = []
                        for j in range(4):
                            h = hg * 4 + j
                            e_sb = ep.tile([128, KSMAX, LMAX], BF16, tag="E")
                            eviews = {}
                            sch = set(range(KS - N_SCH, KS))
                            # interleave Act/DVE kslabs so psum slot drains
                            # alternate engines and the next head's QK can
                            # start before the exp chain finishes
                            acts = [k for k in range(KS) if k not in sch]
                            schs = sorted(sch)
                            order = []
                            for ii in range(max(len(acts), len(schs))):
                                if ii < len(acts):
                                    order.append(acts[ii])
                                if ii < len(schs):
                                    order.append(schs[ii])
                            for ksl in order:
                                sz = min(128, L - ksl * 128)
                                ps = ppool.tile([128, LMAX], F32, tag="big",
                                                padded_shape=[128, 1024],
                                                bufs=3)
                                for (q0, qn) in qch:
                                    nc.tensor.matmul(
                                        ps[0:sz, q0:q0 + qn],
                                        kt[kv_side][32 * j:32 * j + 32, hg, :,
                                                    offs[bl] + ksl * 128:
                                                    offs[bl] + ksl * 128 + sz],
                                        qt[qs_side][32 * j:32 * j + 32, hg, :,
                                                    offs[bl] + q0:
                                                    offs[bl] + q0 + qn],
                                        start=True, stop=True,
                                        perf_mode=DR,
                                        tile_position=(32 * j, 0),
                                    )
                                if ksl in sch:
                                    # Schraudolph exp on DVE; high bf16 half
                                    # of the int32 is read directly by P@V
                                    ei = eip.tile([128, LMAX], I32, tag="ei")
                                    nc.vector.tensor_scalar(
                                        ei[0:sz, 0:L], ps[0:sz, 0:L],
                                        SCH_A,
                                        bias2_sb[0:sz, bl, ksl:ksl + 1],
                                        mybir.AluOpType.mult,
                                        mybir.AluOpType.add,
                                    )
                                    eviews[ksl] = ei.bitcast(BF16).rearrange(
                                        "p (c two) -> p c two", two=2)[:, :, 1]
                                else:
                                    nc.scalar.activation(
                                        e_sb[0:sz, ksl, 0:L], ps[0:sz, 0:L],
                                        EXP,
                                        bias=bias_sb[0:sz, bl, ksl:ksl + 1],
                                        scale=ACT_SCALE,
                                    )
                            pv = ppool.tile([128, QSMAX, DK + 1], F32,
                                            tag="small", bufs=2)
                            for qs in range(QS):
                                # last qslab overlaps: all 128 rows live
                                q0 = qs * 128 if qs < QS - 1 else L - 128
                                for ksl in range(KS):
                                    sz = min(128, L - ksl * 128)
                                    lhsT = (eviews[ksl][0:sz, q0:q0 + 128]
                                            if ksl in eviews else
                                            e_sb[0:sz, ksl, q0:q0 + 128])
                                    nc.tensor.matmul(
                                        pv[:, qs, :],
                                        lhsT,
                                        v_sb[kv_side][0:sz, bl, ksl, h, :],
                                        start=(ksl == 0),
                                        stop=(ksl == KS - 1),
                                    )
                            pvs.append((j, pv))

                            def do_norm(jj, pvv):
                                rc = rp.tile([128, QSMAX, 1], F32, tag="rc",
                                             name="rc")
                                nc.vector.reciprocal(rc[:, 0:QS, :],
                                                     pvv[:, 0:QS, DK:DK + 1])
                                # Pool cannot read PSUM: normalize on DVE
                                nc.vector.tensor_tensor(
                                    out=o_sb[:, 0:QS, jj, :],
                                    in0=pvv[:, 0:QS, 0:DK],
                                    in1=rc[:, 0:QS, :].broadcast_to(
                                        (128, QS, DK)),
                                    op=mybir.AluOpType.mult,
                                )
                            # normalize the PREVIOUS head: DVE issues its
                            # next-head Schraudolph ops without stalling
                            if len(pvs) >= 2:
                                do_norm(*pvs[-2])
                        do_norm(*pvs[-1])
                        nc.sync.dma_start(
                            o_d[att, bl].rearrange(
                                "(q p) d -> p q d",
                                p=128)[:, 0:QS, hg * 256:(hg + 1) * 256],
                            o_sb[:, 0:QS, :, :],
                        )

    _fix_multiwait(nc)
    return nc


def _assign_slots(cnt):
    """Greedy + local-swap assignment of 32 batches into 4 slot-groups of 8,
    minimizing the sum of per-slot expert capacity sums."""
    rng = np.random.default_rng(0)
    Bn = cnt.shape[0]
    best = None
    for _ in range(16):
        perm = rng.permutation(Bn)
        groups = [list(perm[g * 8:(g + 1) * 8]) for g in range(4)]
        improved = True
        it = 0
        while improved and it < 40:
            improved = False
            it += 1
            for g1 in range(4):
                for g2 in range(g1 + 1, 4):
                    for i in range(8):
                        for jj in range(8):
                            c0 = (cnt[groups[g1]].max(axis=0).sum()
                                  + cnt[groups[g2]].max(axis=0).sum())
                            groups[g1][i], groups[g2][jj] = \
                                groups[g2][jj], groups[g1][i]
                            c1 = (cnt[groups[g1]].max(axis=0).sum()
                                  + cnt[groups[g2]].max(axis=0).sum())
                            if c1 < c0:
                                improved = True
                            else:
                                groups[g1][i], groups[g2][jj] = \
                                    groups[g2][jj], groups[g1][i]
        c = sum(cnt[list(g)].max(axis=0).sum() for g in groups)
        if best is None or c < best[0]:
            best = (c, [list(g) for g in groups])
    return best[1]


def _prep_host(hidden1, hidden2, mask, b_seq, W_item, W_type):
    """All host-side routing + layout prep. Returns (meta, in_maps)."""
    fp8 = ml_dtypes.float8_e4m3fn
    cnt = np.zeros((B, NE), dtype=np.int64)
    for e in range(NE):
        cnt[:, e] = (b_seq == e).sum(axis=1)
    groups = _assign_slots(cnt)
    asn = np.array(groups)
    # capacities rounded up to even: DoubleRow ISA needs 2B-aligned offsets
    caps = [((cnt[groups[bl]].max(axis=0).astype(int) + 1) // 2) * 2
            for bl in range(BL)]
    starts = [np.concatenate([[0], np.cumsum(caps[bl])[:-1]]).astype(int)
              for bl in range(BL)]
    Ls = [int(caps[bl].sum()) for bl in range(BL)]
    KSMAX = max(-(-L // 128) for L in Ls)
    offs = np.concatenate([[0], np.cumsum(Ls)]).astype(int)
    TC = int(offs[-1])
    TCP = -(-TC // 16) * 16     # padded stride: DoubleRow needs 16B-aligned

    colmap = np.zeros((B, N), dtype=np.int64)
    for bl in range(BL):
        for g in groups[bl]:
            off = np.zeros(NE, dtype=np.int64)
            for n in range(N):
                e = b_seq[g, n]
                colmap[g, n] = starts[bl][e] + off[e]
                off[e] += 1

    # weights: scaled, fp8 + residual, device layouts
    Wb = [W_item * W_SCALE, W_type * W_SCALE]   # [3, NE, D, H, DK] each
    wqk = np.zeros((2, 2, 2, 2, NE, 2, 128, 4, 128), dtype=fp8)
    wv = np.zeros((2, 4, NE, 2, 128, 4, 128), dtype=fp8)
    for s in range(2):
        W = Wb[s]
        # Q/K: [NE, D, H, DK] -> (e, ks, p, hg, j, tk, dkm)
        for i in range(2):
            w = W[i].reshape(NE, 4, 128, 2, 4, 2, 32)
            # -> [hg, tk, e, p, ks, (j, dkm)]
            w = w.transpose(3, 5, 0, 2, 1, 4, 6).reshape(
                2, 2, NE, 128, 4, 128)
            w8 = w.astype(fp8)
            wr8 = (w - w8.astype(np.float32)).astype(fp8)
            wqk[s, i, :, :, :, 0] = w8
            wqk[s, i, :, :, :, 1] = wr8
        # V: [NE, D, 512] -> (ms, e, p, ks, m)
        w = W[2].reshape(NE, 4, 128, 4, 128).transpose(3, 0, 2, 1, 4)
        w8 = w.astype(fp8)
        wr8 = (w - w8.astype(np.float32)).astype(fp8)
        wv[s, :, :, 0] = w8
        wv[s, :, :, 1] = wr8
    iden = np.eye(128, dtype=np.float32).astype(ml_dtypes.bfloat16)

    in_maps = []
    for c in range(NCORES):
        xs = []
        for hid in (hidden1, hidden2):
            x = np.zeros((D, TCP), dtype=np.float32)
            for bl in range(BL):
                g = asn[bl][c]
                x[:, offs[bl] + colmap[g]] = hid[g].T
            x = x.reshape(4, 128, TCP).transpose(1, 0, 2)
            x8 = x.astype(fp8)
            xr8 = (x - x8.astype(np.float32)).astype(fp8)
            xs.append((x8, xr8))
        biasp = np.full((128, BL, KSMAX), -10000.0, dtype=np.float32)
        bias2p = np.full((128, BL, KSMAX), SCH_B + SCH_BIAS_PAD,
                         dtype=np.float32)
        for bl in range(BL):
            g = asn[bl][c]
            real = colmap[g][mask[g]]
            biasp[real % 128, bl, real // 128] = 0.0
            bias2p[real % 128, bl, real // 128] = SCH_B
        in_maps.append({
            "x1": xs[0][0], "xr1": xs[0][1],
            "x2": xs[1][0], "xr2": xs[1][1],
            "wqk0": wqk[0], "wqk1": wqk[1],
            "wv0": wv[0], "wv1": wv[1],
            "bias": biasp, "bias2": bias2p, "iden": iden,
        })
    meta = dict(groups=groups, asn=asn, caps=caps, starts=starts, Ls=Ls,
                colmap=colmap, offs=offs, TC=TC)
    return meta, in_maps


def kernel(hidden1, hidden2, mask, b_seq, W_item, W_type):
    hidden1 = np.asarray(hidden1, dtype=np.float32)
    hidden2 = np.asarray(hidden2, dtype=np.float32)
    mask = np.asarray(mask).astype(bool)
    b_seq = np.asarray(b_seq, dtype=np.int32)
    W_item = np.asarray(W_item, dtype=np.float32)
    W_type = np.asarray(W_type, dtype=np.float32)

    meta, in_maps = _prep_host(hidden1, hidden2, mask, b_seq, W_item, W_type)
    caps, starts, Ls = meta["caps"], meta["starts"], meta["Ls"]
    asn, colmap = meta["asn"], meta["colmap"]

    nc = _build_program(caps, starts, Ls)
    res = run_bass_kernel_spmd(nc, in_maps, list(range(NCORES)))

    # --- unshard: un-permute tokens (last qslab overlaps: remap rows) ---
    h1 = np.zeros((B, N, D), dtype=np.float32)
    h2 = np.zeros((B, N, D), dtype=np.float32)
    rowmaps = []
    for bl in range(BL):
        L = Ls[bl]
        QS = -(-L // 128)
        rm = np.arange(L)
        if QS > 1:
            hi = rm >= (QS - 1) * 128
            rm[hi] = rm[hi] + QS * 128 - L
        rowmaps.append(rm)
    for c in range(NCORES):
        o = np.asarray(res.results[c]["o"]).astype(np.float32)
        for bl in range(BL):
            g = asn[bl][c]
            rows = rowmaps[bl][colmap[g]]
            h1[g] = o[0, bl][rows]
            h2[g] = o[1, bl][rows]
    return (h1, h2)
